# revision 58
# baseline (speedup 1.0000x reference)
"""Trainium2 Bass kernel for CustomCrossEntropyLoss (data-parallel over 8 NeuronCores).

Math (matches the reference):
    mask  = (target != 4)
    lse   = logsumexp(pred, axis=C)        # no max-subtraction: pred ~ N(0,1)
    p_t   = pred[target]   (raw-pred one-hot gather; 0 on ignored pixels)
    w     = 1.0 where ME == 0 else 0.5
    loss  = sum(w * mask * (lse - p_t)) / sum(mask)

Engine split per core (2 batches = 2 groups of [128, 2048] per class plane):
  DMA    : ONE need-ordered sync HWDGE queue carries everything as raw
           dtypes (t/ME stay int32 - engine ALUs are fp32-exact <=4; the
           SWDGE casting path is 2x slower and was the original bottleneck).
           Half-plane granularity halves the per-DMA completion-sem lag.
  ACT    : m = Sign(4-t) (+accum_out -> count), e_c = exp(pred_c) -> bf16,
           lse = Ln(Se) from PSUM.
  DVE    : y_c = (t==c)*pred_c one fused STT per half-plane (STT is always
           1x, so fp32 in1 is free), lsem = m*lse (TT 2x), weighted sums
           A += (ME-2)*lsem and B += (ME-2)*p_t via STT accum_out.  This is
           the critical-path engine; everything movable is moved off it.
  PE     : Se = sum_c e_c and p_t = sum_c y_c as identity-lhsT
           PSUM-accumulated matmuls (bf16 rhs, 1 cyc/row), se before pt.
  GpSimd : idle (shares the DVE SBUF port; using it stalls the DVE).

  B needs no mask (p_t == 0 on ignored pixels); only the lse side is
  masked.  Host: loss = -(A - B) / (2 * count).
"""
import sys

sys.path.insert(0, "/opt/trn_rl_repo")

import numpy as np
from contextlib import ExitStack

import concourse.bacc as bacc
import concourse.tile as tile
from concourse import mybir
from concourse.bass_utils import run_bass_kernel_spmd

N_CORES = 8
B, C, H, W = 16, 4, 512, 512
HW = H * W                      # 262144 pixels per batch
BPC = B // N_CORES              # 2 batches per core
NG = BPC                        # 2 groups per core (one batch each)
F = HW // 128                   # 2048 free-dim columns per group
FH = F // 2                     # 1024-column halves
A = mybir.AluOpType
DT = mybir.dt
ACTF = mybir.ActivationFunctionType

# stats columns: [A x5][B x5][count(g) x2] (last half splits into 2 cols)
COL_A = 0
COL_B = 5
COL_CNT = 10
NSTAT = 12

_nc_cache = None


def _build():
    nc = bacc.Bacc()
    pred = nc.dram_tensor("pred", [BPC, C, HW], DT.float32, kind="ExternalInput")
    targ = nc.dram_tensor("targ", [BPC, HW], DT.int32, kind="ExternalInput")
    me = nc.dram_tensor("me", [BPC, HW], DT.int32, kind="ExternalInput")
    identd = nc.dram_tensor("identd", [128, 128], DT.bfloat16, kind="ExternalInput")
    stats = nc.dram_tensor("stats", [128, NSTAT], DT.float32, kind="ExternalOutput")

    with tile.TileContext(nc) as tc:
        with ExitStack() as ctx:
            big = ctx.enter_context(tc.tile_pool(name="big", bufs=2))
            mid = ctx.enter_context(tc.tile_pool(name="mid", bufs=2))
            psum = ctx.enter_context(tc.tile_pool(name="psum", bufs=2, space="PSUM"))
            singles = ctx.enter_context(tc.tile_pool(name="singles", bufs=1))

            stats_t = singles.tile([128, NSTAT], DT.float32)
            four_t = singles.tile([128, 1], DT.float32)
            nc.vector.memset(four_t, 4.0)


            # identity [128,128] bf16 lhsT: shipped from the host (avoids the
            # gpsimd iota + library-load in the startup critical path)
            ident = singles.tile([128, 128], DT.bfloat16)

            # ---- input DMAs (program order per queue) ----
            # sync queue carries ONLY pred (8 MB) so its first planes land
            # ASAP and it drains ~31us.  Everything else (ident, t, ME as raw
            # int32 - ACT/DVE read int32 directly, fp32 ALUs are exact <=4)
            # rides the scalar HWDGE queue.  t0 in halves so the first y STT
            # can start early; g1's last class plane in halves to shrink the
            # tail chain exp->matmul->Ln->A.
            t_bf, me_bf, p_t = [], [], []
            for g in range(NG):
                tb = big.tile([128, F], DT.int32, tag="tbf", name=f"tbf{g}")
                t_bf.append(tb)
                mb = big.tile([128, F], DT.int32, tag="mebf", name=f"mebf{g}")
                me_bf.append(mb)
            # scalar queue: just the tiny identity (zero contention)
            nc.scalar.dma_start(out=ident, in_=identd[:, :])
            # single sync queue, ordered by consumption time: one queue
            # sustains the same ~400 GB/s as two, and the order IS the
            # schedule.
            t_src = [targ[g, :].rearrange("(q n) -> q n", n=F) for g in range(NG)]
            me_src = [me[g, :].rearrange("(q n) -> q n", n=F) for g in range(NG)]
            p_srcs = [
                [pred[g, c, :].rearrange("(q n) -> q n", n=F) for c in range(C)]
                for g in range(NG)
            ]
            for g in range(NG):
                p_t.append(
                    big.tile([128, C, F], DT.float32, tag="p", name=f"p{g}")
                )

            # every pred plane in halves: halves the per-DMA completion
            # semaphore lag and doubles pipeline granularity
            def dma_p(g, c):
                for h in range(2):
                    nc.sync.dma_start(
                        out=p_t[g][:, c, FH * h : FH * (h + 1)],
                        in_=p_srcs[g][c][:, FH * h : FH * (h + 1)],
                    )

            for h in range(2):
                nc.sync.dma_start(
                    out=t_bf[0][:, FH * h : FH * (h + 1)],
                    in_=t_src[0][:, FH * h : FH * (h + 1)],
                )
            dma_p(0, 0)
            dma_p(0, 1)
            dma_p(0, 2)
            dma_p(0, 3)
            nc.sync.dma_start(out=t_bf[1], in_=t_src[1])
            dma_p(1, 0)
            dma_p(1, 1)
            # me0/me1 ride late: only the A/B reductions read them
            nc.sync.dma_start(out=me_bf[0], in_=me_src[0])
            dma_p(1, 2)
            nc.sync.dma_start(out=me_bf[1], in_=me_src[1])
            # last plane: half then two quarters (shortest completion lag
            # on the bytes that gate the tail chain)
            nc.sync.dma_start(
                out=p_t[1][:, 3, 0:FH], in_=p_srcs[1][3][:, 0:FH]
            )
            for q0, q1 in ((FH, FH + 512), (FH + 512, F)):
                nc.sync.dma_start(
                    out=p_t[1][:, 3, q0:q1], in_=p_srcs[1][3][:, q0:q1]
                )

            # ---- per-group compute ----
            # p_t is 0 on ignored pixels, so B = sum (ME-2)*p_t needs no
            # mask; only the lse side is masked (lsem = m*lse).
            m_t, e_t, y_t = [], [], []
            for g in range(NG):
                # mask plane + count on ACT: m = sign(4 - t) in {0,1}
                # (ACT has slack; DVE is the critical path)
                m_ = mid.tile([128, F], DT.bfloat16, tag="m", name=f"m{g}")
                nc.scalar.activation(
                    out=m_, in_=t_bf[g], func=ACTF.Sign, scale=-1.0, bias=four_t,
                    accum_out=stats_t[:, COL_CNT + g : COL_CNT + g + 1],
                )
                m_t.append(m_)
                e_ = mid.tile([128, C, F], DT.bfloat16, tag="e", name=f"e{g}")
                y_ = mid.tile([128, C, F], DT.bfloat16, tag="y", name=f"y{g}")
                for c in range(C):
                    # quarter-granularity on the tail half of the last plane
                    tail = g == NG - 1 and c == C - 1
                    bounds = [0, FH, F] if not tail else [0, FH, FH + 512, F]
                    for b0, b1 in zip(bounds[:-1], bounds[1:]):
                        sl = slice(b0, b1)
                        nc.scalar.activation(
                            out=e_[:, c, sl], in_=p_t[g][:, c, sl], func=ACTF.Exp
                        )
                        nc.vector.scalar_tensor_tensor(
                            out=y_[:, c, sl], in0=t_bf[g][:, sl],
                            scalar=float(c), in1=p_t[g][:, c, sl],
                            op0=A.is_equal, op1=A.mult,
                        )
                e_t.append(e_)
                y_t.append(y_)

            col_a = COL_A
            col_b = COL_B
            for g in range(NG):
                for h in range(2):
                    se_ps = psum.tile([128, FH], DT.float32, tag="se", name=f"se{g}{h}")
                    pt_ps = psum.tile([128, FH], DT.float32, tag="pt", name=f"pt{g}{h}")
                    # se before pt: Ln (-> lsem -> A) is the longer chain
                    for k in range(FH // 512):
                        sl = slice(FH * h + 512 * k, FH * h + 512 * (k + 1))
                        pl = slice(512 * k, 512 * (k + 1))
                        for c in range(C):
                            nc.tensor.matmul(
                                out=se_ps[:, pl], lhsT=ident, rhs=e_t[g][:, c, sl],
                                start=(c == 0), stop=(c == C - 1),
                            )
                    for k in range(FH // 512):
                        sl = slice(FH * h + 512 * k, FH * h + 512 * (k + 1))
                        pl = slice(512 * k, 512 * (k + 1))
                        for c in range(C):
                            nc.tensor.matmul(
                                out=pt_ps[:, pl], lhsT=ident, rhs=y_t[g][:, c, sl],
                                start=(c == 0), stop=(c == C - 1),
                            )
                    # reduce chain width: 512 on the very last half so the
                    # serial tail (Ln -> lsem -> A) is as short as possible
                    W = 512 if (g == NG - 1 and h == 1) else FH
                    for j in range(FH // W):
                        hsl = slice(FH * h + W * j, FH * h + W * (j + 1))
                        pl = slice(W * j, W * (j + 1))
                        # B += (ME-2) * p_t  (no mask; p_t==0 when ignored)
                        dumB = mid.tile(
                            [128, W], DT.bfloat16, tag="dumB", name=f"dB{g}{h}{j}",
                            padded_shape=[128, FH],
                        )
                        nc.vector.scalar_tensor_tensor(
                            out=dumB, in0=me_bf[g][:, hsl], scalar=2.0,
                            in1=pt_ps[:, pl], op0=A.subtract, op1=A.mult,
                            accum_out=stats_t[:, col_b : col_b + 1],
                        )
                        col_b += 1
                        lse = mid.tile(
                            [128, W], DT.bfloat16, tag="lse", name=f"lse{g}{h}{j}",
                            padded_shape=[128, FH],
                        )
                        nc.scalar.activation(out=lse, in_=se_ps[:, pl], func=ACTF.Ln)
                        # lsem = m * lse on DVE (2x TT; gpsimd shares the DVE
                        # SBUF port and stalls both when DVE is busy)
                        lsem = mid.tile(
                            [128, W], DT.bfloat16, tag="lsem", name=f"lm{g}{h}{j}",
                            padded_shape=[128, FH],
                        )
                        nc.vector.tensor_tensor(
                            out=lsem, in0=m_t[g][:, hsl], in1=lse, op=A.mult
                        )
                        dumA = mid.tile(
                            [128, W], DT.bfloat16, tag="dumA", name=f"dA{g}{h}{j}",
                            padded_shape=[128, FH],
                        )
                        nc.vector.scalar_tensor_tensor(
                            out=dumA, in0=me_bf[g][:, hsl], scalar=2.0, in1=lsem,
                            op0=A.subtract, op1=A.mult,
                            accum_out=stats_t[:, col_a : col_a + 1],
                        )
                        col_a += 1

            nc.sync.dma_start(out=stats[:, :], in_=stats_t)
    nc.finalize()
    return nc


def _get_nc():
    global _nc_cache
    if _nc_cache is None:
        _nc_cache = _build()
    return _nc_cache


def _install_ntff_hook():
    """Register the axon NTFF profiling hook (missing antenv.axon_hooks glue)."""
    import types
    import ctypes
    import contextlib

    try:
        from antenv.axon_hooks import get_axon_ntff_profile_hook  # noqa: F401

        return
    except ImportError:
        pass

    so_path = "/opt/axon/libaxon_pjrt.so"
    try:
        lib = ctypes.CDLL(so_path)
    except OSError:
        return
    if not hasattr(lib, "axon_start_nrt_profile"):
        return
    lib.axon_start_nrt_profile.argtypes = [
        ctypes.POINTER(ctypes.c_int64),
        ctypes.c_size_t,
    ]
    lib.axon_start_nrt_profile.restype = ctypes.c_int64
    lib.axon_stop_nrt_profile.argtypes = [ctypes.c_char_p]
    lib.axon_stop_nrt_profile.restype = ctypes.c_int64

    @contextlib.contextmanager
    def _hook(output_dir, device_ids):
        import jax

        jax.devices()
        if device_ids:
            ids = (ctypes.c_int64 * len(device_ids))(*device_ids)
            rc = lib.axon_start_nrt_profile(ids, len(device_ids))
        else:
            rc = lib.axon_start_nrt_profile(None, 0)
        if rc != 0:
            raise RuntimeError(f"axon_start_nrt_profile rc={rc}")
        try:
            yield
        finally:
            n = lib.axon_stop_nrt_profile(str(output_dir).encode())
            print(f"ntff profile: {n} file(s) -> {output_dir}")

    mod = types.ModuleType("antenv.axon_hooks")
    mod.get_axon_ntff_profile_hook = lambda: _hook
    mod.set_axon_ntff_profile_hook = lambda h: None
    sys.modules["antenv.axon_hooks"] = mod

    from concourse import bass_utils as _bu

    _bu.upload_artifacts = lambda tmpdir: tmpdir


def _run(pred, target, ME, trace=False, tmpdir=None):
    pred = np.ascontiguousarray(pred, dtype=np.float32).reshape(B, C, HW)
    target = np.ascontiguousarray(target, dtype=np.int32).reshape(B, HW)
    ME = np.ascontiguousarray(ME, dtype=np.int32).reshape(B, HW)

    import ml_dtypes

    ident_np = np.eye(128, dtype=ml_dtypes.bfloat16)
    in_maps = []
    for i in range(N_CORES):
        sl = slice(i * BPC, (i + 1) * BPC)
        in_maps.append(
            {
                "pred": np.ascontiguousarray(pred[sl]),
                "targ": np.ascontiguousarray(target[sl]),
                "me": np.ascontiguousarray(ME[sl]),
                "identd": ident_np,
            }
        )

    nc = _get_nc()
    if trace:
        _install_ntff_hook()
    res = run_bass_kernel_spmd(
        nc, in_maps, core_ids=list(range(N_CORES)), trace=trace, tmpdir=tmpdir
    )

    acc_a = acc_b = cnt = 0.0
    for i in range(N_CORES):
        st = res.results[i]["stats"].astype(np.float64)
        acc_a += st[:, COL_A : COL_A + 5].sum()
        acc_b += st[:, COL_B : COL_B + 5].sum()
        cnt += st[:, COL_CNT : COL_CNT + 2].sum()

    # wm2 = (ME-2)*mask = -2*w*mask  =>  sum(w*mask*(lse-p_t)) = -(A-B)/2
    loss = -(acc_a - acc_b) / (2.0 * cnt)
    return np.float32(loss), res.exec_time_ns


def kernel(pred, target, ME):
    loss, _ = _run(pred, target, ME, trace=False)
    return loss


# revision 59
# speedup vs baseline: 1.0921x; 1.0921x over previous
"""Trainium2 Bass kernel for CustomCrossEntropyLoss (data-parallel over 8 NeuronCores).

Math (matches the reference):
    mask  = (target != 4)
    lse   = logsumexp(pred, axis=C)        # no max-subtraction: pred ~ N(0,1)
    p_t   = pred[target]   (raw-pred one-hot gather; 0 on ignored pixels)
    w     = 1.0 where ME == 0 else 0.5
    loss  = sum(w * mask * (lse - p_t)) / sum(mask)

Engine split per core (2 batches = 2 groups of [128, 2048] per class plane):
  DMA    : ONE need-ordered sync HWDGE queue carries everything as raw
           dtypes (t/ME stay int32 - engine ALUs are fp32-exact <=4; the
           SWDGE casting path is 2x slower and was the original bottleneck).
           Half-plane granularity halves the per-DMA completion-sem lag.
  ACT    : m = Sign(4-t) (+accum_out -> count), e_c = exp(pred_c) -> bf16,
           lse = Ln(Se) from PSUM.
  DVE    : y_c = (t==c)*pred_c one fused STT per half-plane (STT is always
           1x, so fp32 in1 is free), lsem = m*lse (TT 2x), weighted sums
           A += (ME-2)*lsem and B += (ME-2)*p_t via STT accum_out.  This is
           the critical-path engine; everything movable is moved off it.
  PE     : Se = sum_c e_c and p_t = sum_c y_c as identity-lhsT
           PSUM-accumulated matmuls (bf16 rhs, 1 cyc/row), se before pt.
  GpSimd : idle (shares the DVE SBUF port; using it stalls the DVE).

  B needs no mask (p_t == 0 on ignored pixels); only the lse side is
  masked.  Host: loss = -(A - B) / (2 * count).
"""
import sys

sys.path.insert(0, "/opt/trn_rl_repo")

import numpy as np
from contextlib import ExitStack

import concourse.bacc as bacc
import concourse.tile as tile
from concourse import mybir
from concourse.bass_utils import run_bass_kernel_spmd

N_CORES = 8
B, C, H, W = 16, 4, 512, 512
HW = H * W                      # 262144 pixels per batch
BPC = B // N_CORES              # 2 batches per core
NG = BPC                        # 2 groups per core (one batch each)
F = HW // 128                   # 2048 free-dim columns per group
FH = F // 2                     # 1024-column halves
A = mybir.AluOpType
DT = mybir.dt
ACTF = mybir.ActivationFunctionType

# stats columns: [A x5][B x5][count(g) x2] (last half splits into 2 cols)
COL_A = 0
COL_B = 5
COL_CNT = 10
NSTAT = 12

_nc_cache = None


def _build():
    nc = bacc.Bacc()
    pred = nc.dram_tensor("pred", [BPC, C, HW], DT.float32, kind="ExternalInput")
    targ = nc.dram_tensor("targ", [BPC, HW], DT.int32, kind="ExternalInput")
    me = nc.dram_tensor("me", [BPC, HW], DT.int32, kind="ExternalInput")
    identd = nc.dram_tensor("identd", [128, 128], DT.bfloat16, kind="ExternalInput")
    stats = nc.dram_tensor("stats", [128, NSTAT], DT.float32, kind="ExternalOutput")

    with tile.TileContext(nc) as tc:
        with ExitStack() as ctx:
            big = ctx.enter_context(tc.tile_pool(name="big", bufs=2))
            mid = ctx.enter_context(tc.tile_pool(name="mid", bufs=2))
            psum = ctx.enter_context(tc.tile_pool(name="psum", bufs=2, space="PSUM"))
            singles = ctx.enter_context(tc.tile_pool(name="singles", bufs=1))

            stats_t = singles.tile([128, NSTAT], DT.float32)
            four_t = singles.tile([128, 1], DT.float32)
            nc.vector.memset(four_t, 4.0)


            # identity [128,128] bf16 lhsT: shipped from the host (avoids the
            # gpsimd iota + library-load in the startup critical path)
            ident = singles.tile([128, 128], DT.bfloat16)

            # ---- input DMAs (program order per queue) ----
            # sync queue carries ONLY pred (8 MB) so its first planes land
            # ASAP and it drains ~31us.  Everything else (ident, t, ME as raw
            # int32 - ACT/DVE read int32 directly, fp32 ALUs are exact <=4)
            # rides the scalar HWDGE queue.  t0 in halves so the first y STT
            # can start early; g1's last class plane in halves to shrink the
            # tail chain exp->matmul->Ln->A.
            t_bf, me_bf, p_t = [], [], []
            for g in range(NG):
                tb = big.tile([128, F], DT.int32, tag="tbf", name=f"tbf{g}")
                t_bf.append(tb)
                mb = big.tile([128, F], DT.int32, tag="mebf", name=f"mebf{g}")
                me_bf.append(mb)
            # scalar queue: just the tiny identity (zero contention)
            nc.scalar.dma_start(out=ident, in_=identd[:, :])
            # single sync queue, ordered by consumption time: one queue
            # sustains the same ~400 GB/s as two, and the order IS the
            # schedule.
            t_src = [targ[g, :].rearrange("(q n) -> q n", n=F) for g in range(NG)]
            me_src = [me[g, :].rearrange("(q n) -> q n", n=F) for g in range(NG)]
            p_srcs = [
                [pred[g, c, :].rearrange("(q n) -> q n", n=F) for c in range(C)]
                for g in range(NG)
            ]
            for g in range(NG):
                p_t.append(
                    big.tile([128, C, F], DT.float32, tag="p", name=f"p{g}")
                )

            # every pred plane in halves: halves the per-DMA completion
            # semaphore lag and doubles pipeline granularity
            def dma_p(g, c):
                for h in range(2):
                    nc.sync.dma_start(
                        out=p_t[g][:, c, FH * h : FH * (h + 1)],
                        in_=p_srcs[g][c][:, FH * h : FH * (h + 1)],
                    )

            for h in range(2):
                nc.sync.dma_start(
                    out=t_bf[0][:, FH * h : FH * (h + 1)],
                    in_=t_src[0][:, FH * h : FH * (h + 1)],
                )
            dma_p(0, 0)
            dma_p(0, 1)
            dma_p(0, 2)
            dma_p(0, 3)
            nc.sync.dma_start(out=t_bf[1], in_=t_src[1])
            dma_p(1, 0)
            dma_p(1, 1)
            # me0/me1 ride late: only the A/B reductions read them
            nc.sync.dma_start(out=me_bf[0], in_=me_src[0])
            dma_p(1, 2)
            # last plane: half then two quarters (shortest completion lag
            # on the bytes that gate the tail chain); me1 rides dead last —
            # its consumers (A/B of g1) run after the pt matmuls anyway
            nc.sync.dma_start(
                out=p_t[1][:, 3, 0:FH], in_=p_srcs[1][3][:, 0:FH]
            )
            for q0, q1 in ((FH, FH + 512), (FH + 512, F)):
                nc.sync.dma_start(
                    out=p_t[1][:, 3, q0:q1], in_=p_srcs[1][3][:, q0:q1]
                )
            nc.sync.dma_start(out=me_bf[1], in_=me_src[1])

            # ---- per-group compute ----
            # p_t is 0 on ignored pixels, so B = sum (ME-2)*p_t needs no
            # mask; only the lse side is masked (lsem = m*lse).
            m_t, e_t, y_t = [], [], []
            for g in range(NG):
                # mask plane + count on ACT: m = sign(4 - t) in {0,1}
                # (ACT has slack; DVE is the critical path)
                m_ = mid.tile([128, F], DT.bfloat16, tag="m", name=f"m{g}")
                nc.scalar.activation(
                    out=m_, in_=t_bf[g], func=ACTF.Sign, scale=-1.0, bias=four_t,
                    accum_out=stats_t[:, COL_CNT + g : COL_CNT + g + 1],
                )
                m_t.append(m_)
                e_ = mid.tile([128, C, F], DT.bfloat16, tag="e", name=f"e{g}")
                y_ = mid.tile([128, C, F], DT.bfloat16, tag="y", name=f"y{g}")
                for c in range(C):
                    # quarter-granularity on the tail half of the last plane
                    tail = g == NG - 1 and c == C - 1
                    bounds = [0, FH, F] if not tail else [0, FH, FH + 512, F]
                    for b0, b1 in zip(bounds[:-1], bounds[1:]):
                        sl = slice(b0, b1)
                        nc.scalar.activation(
                            out=e_[:, c, sl], in_=p_t[g][:, c, sl], func=ACTF.Exp
                        )
                        nc.vector.scalar_tensor_tensor(
                            out=y_[:, c, sl], in0=t_bf[g][:, sl],
                            scalar=float(c), in1=p_t[g][:, c, sl],
                            op0=A.is_equal, op1=A.mult,
                        )
                e_t.append(e_)
                y_t.append(y_)

            col_a = COL_A
            col_b = COL_B
            for g in range(NG):
                for h in range(2):
                    se_ps = psum.tile([128, FH], DT.float32, tag="se", name=f"se{g}{h}")
                    pt_ps = psum.tile([128, FH], DT.float32, tag="pt", name=f"pt{g}{h}")
                    # se before pt: Ln (-> lsem -> A) is the longer chain
                    for k in range(FH // 512):
                        sl = slice(FH * h + 512 * k, FH * h + 512 * (k + 1))
                        pl = slice(512 * k, 512 * (k + 1))
                        for c in range(C):
                            nc.tensor.matmul(
                                out=se_ps[:, pl], lhsT=ident, rhs=e_t[g][:, c, sl],
                                start=(c == 0), stop=(c == C - 1),
                            )
                    for k in range(FH // 512):
                        sl = slice(FH * h + 512 * k, FH * h + 512 * (k + 1))
                        pl = slice(512 * k, 512 * (k + 1))
                        for c in range(C):
                            nc.tensor.matmul(
                                out=pt_ps[:, pl], lhsT=ident, rhs=y_t[g][:, c, sl],
                                start=(c == 0), stop=(c == C - 1),
                            )
                    # reduce chain width: 512 on the very last half so the
                    # serial tail (Ln -> lsem -> A) is as short as possible
                    W = 512 if (g == NG - 1 and h == 1) else FH
                    for j in range(FH // W):
                        hsl = slice(FH * h + W * j, FH * h + W * (j + 1))
                        pl = slice(W * j, W * (j + 1))
                        # B += (ME-2) * p_t  (no mask; p_t==0 when ignored)
                        dumB = mid.tile(
                            [128, W], DT.bfloat16, tag="dumB", name=f"dB{g}{h}{j}",
                            padded_shape=[128, FH],
                        )
                        nc.vector.scalar_tensor_tensor(
                            out=dumB, in0=me_bf[g][:, hsl], scalar=2.0,
                            in1=pt_ps[:, pl], op0=A.subtract, op1=A.mult,
                            accum_out=stats_t[:, col_b : col_b + 1],
                        )
                        col_b += 1
                        lse = mid.tile(
                            [128, W], DT.bfloat16, tag="lse", name=f"lse{g}{h}{j}",
                            padded_shape=[128, FH],
                        )
                        nc.scalar.activation(out=lse, in_=se_ps[:, pl], func=ACTF.Ln)
                        # lsem = m * lse on DVE (2x TT; gpsimd shares the DVE
                        # SBUF port and stalls both when DVE is busy)
                        lsem = mid.tile(
                            [128, W], DT.bfloat16, tag="lsem", name=f"lm{g}{h}{j}",
                            padded_shape=[128, FH],
                        )
                        nc.vector.tensor_tensor(
                            out=lsem, in0=m_t[g][:, hsl], in1=lse, op=A.mult
                        )
                        dumA = mid.tile(
                            [128, W], DT.bfloat16, tag="dumA", name=f"dA{g}{h}{j}",
                            padded_shape=[128, FH],
                        )
                        nc.vector.scalar_tensor_tensor(
                            out=dumA, in0=me_bf[g][:, hsl], scalar=2.0, in1=lsem,
                            op0=A.subtract, op1=A.mult,
                            accum_out=stats_t[:, col_a : col_a + 1],
                        )
                        col_a += 1

            nc.sync.dma_start(out=stats[:, :], in_=stats_t)
    nc.finalize()
    return nc


def _get_nc():
    global _nc_cache
    if _nc_cache is None:
        _nc_cache = _build()
    return _nc_cache


def _install_ntff_hook():
    """Register the axon NTFF profiling hook (missing antenv.axon_hooks glue)."""
    import types
    import ctypes
    import contextlib

    try:
        from antenv.axon_hooks import get_axon_ntff_profile_hook  # noqa: F401

        return
    except ImportError:
        pass

    so_path = "/opt/axon/libaxon_pjrt.so"
    try:
        lib = ctypes.CDLL(so_path)
    except OSError:
        return
    if not hasattr(lib, "axon_start_nrt_profile"):
        return
    lib.axon_start_nrt_profile.argtypes = [
        ctypes.POINTER(ctypes.c_int64),
        ctypes.c_size_t,
    ]
    lib.axon_start_nrt_profile.restype = ctypes.c_int64
    lib.axon_stop_nrt_profile.argtypes = [ctypes.c_char_p]
    lib.axon_stop_nrt_profile.restype = ctypes.c_int64

    @contextlib.contextmanager
    def _hook(output_dir, device_ids):
        import jax

        jax.devices()
        if device_ids:
            ids = (ctypes.c_int64 * len(device_ids))(*device_ids)
            rc = lib.axon_start_nrt_profile(ids, len(device_ids))
        else:
            rc = lib.axon_start_nrt_profile(None, 0)
        if rc != 0:
            raise RuntimeError(f"axon_start_nrt_profile rc={rc}")
        try:
            yield
        finally:
            n = lib.axon_stop_nrt_profile(str(output_dir).encode())
            print(f"ntff profile: {n} file(s) -> {output_dir}")

    mod = types.ModuleType("antenv.axon_hooks")
    mod.get_axon_ntff_profile_hook = lambda: _hook
    mod.set_axon_ntff_profile_hook = lambda h: None
    sys.modules["antenv.axon_hooks"] = mod

    from concourse import bass_utils as _bu

    _bu.upload_artifacts = lambda tmpdir: tmpdir


def _run(pred, target, ME, trace=False, tmpdir=None):
    pred = np.ascontiguousarray(pred, dtype=np.float32).reshape(B, C, HW)
    target = np.ascontiguousarray(target, dtype=np.int32).reshape(B, HW)
    ME = np.ascontiguousarray(ME, dtype=np.int32).reshape(B, HW)

    import ml_dtypes

    ident_np = np.eye(128, dtype=ml_dtypes.bfloat16)
    in_maps = []
    for i in range(N_CORES):
        sl = slice(i * BPC, (i + 1) * BPC)
        in_maps.append(
            {
                "pred": np.ascontiguousarray(pred[sl]),
                "targ": np.ascontiguousarray(target[sl]),
                "me": np.ascontiguousarray(ME[sl]),
                "identd": ident_np,
            }
        )

    nc = _get_nc()
    if trace:
        _install_ntff_hook()
    res = run_bass_kernel_spmd(
        nc, in_maps, core_ids=list(range(N_CORES)), trace=trace, tmpdir=tmpdir
    )

    acc_a = acc_b = cnt = 0.0
    for i in range(N_CORES):
        st = res.results[i]["stats"].astype(np.float64)
        acc_a += st[:, COL_A : COL_A + 5].sum()
        acc_b += st[:, COL_B : COL_B + 5].sum()
        cnt += st[:, COL_CNT : COL_CNT + 2].sum()

    # wm2 = (ME-2)*mask = -2*w*mask  =>  sum(w*mask*(lse-p_t)) = -(A-B)/2
    loss = -(acc_a - acc_b) / (2.0 * cnt)
    return np.float32(loss), res.exec_time_ns


def kernel(pred, target, ME):
    loss, _ = _run(pred, target, ME, trace=False)
    return loss
